# revision 1
# baseline (speedup 1.0000x reference)
"""Trainium2 Bass kernel for CustomLSTM: B=64, T=1024, I=H=512.

Sharding: data-parallel over batch, 8 sequences per core on 8 cores.
Everything on-device lives in TRANSPOSED layout (hidden/gate dim on SBUF
partitions, batch on the free dim) so the per-step elementwise chain runs on
all 128 lanes and h^T feeds the next step's matmul directly, zero transposes.

Phase 1 (per core): xwT[g, t, b] = (x @ W + bias)^T via float32r matmuls
  (full-rate: moving free dim 512), staged to DRAM scratch.
Phase 2: 1024 sequential steps. gates^T = U-tiles (stationary, bf16)
  @ h^T (moving, N=8), PSUM-accumulated over 4 K-tiles; sigmoid/tanh on ACT,
  muls on DVE; h^T written back to SBUF state and staged out.
"""

import numpy as np
import ml_dtypes

B, T, I, H = 64, 1024, 512, 512
NC = 8            # cores
BL = B // NC      # 8 sequences per core
G4 = 4 * H        # 2048 gate dim
KT = I // 128     # 4 contraction tiles
MT = G4 // 128    # 16 gate m-tiles
C = T * BL        # 8192 columns, col = t*8 + b
MACRO = 64        # timesteps per For_i iteration
CHUNK = 64        # timesteps per precompute chunk (512 columns)


def build(nc, bass, tile, mybir):
    f32, bf16, f32r = mybir.dt.float32, mybir.dt.bfloat16, mybir.dt.float32r
    AF = mybir.ActivationFunctionType

    xT = nc.dram_tensor("xT", [128, KT, C], bf16, kind="ExternalInput")
    W = nc.dram_tensor("W", [128, KT, G4], bf16, kind="ExternalInput")
    U = nc.dram_tensor("U", [128, KT, G4], bf16, kind="ExternalInput")
    biasT = nc.dram_tensor("biasT", [128, MT], f32, kind="ExternalInput")
    hT_out = nc.dram_tensor("hT_out", [128, KT, C], f32, kind="ExternalOutput")

    with tile.TileContext(nc) as tc:
        with (
            tc.tile_pool(name="const", bufs=1) as const,
            tc.tile_pool(name="xtc", bufs=2) as xtc_pool,
            tc.tile_pool(name="xwc", bufs=3) as xwc_pool,
            tc.tile_pool(name="pre_ps", bufs=2, space="PSUM") as pre_ps,
            tc.tile_pool(name="state", bufs=1) as state,
            tc.tile_pool(name="xw", bufs=2) as xw_pool,
            tc.tile_pool(name="g_ps", bufs=2, space="PSUM") as g_ps,
            tc.tile_pool(name="work", bufs=2) as work,
            tc.tile_pool(name="stage", bufs=2) as stage_pool,
            tc.tile_pool(name="dram", bufs=1, space="DRAM") as dram,
        ):
            W_sb = const.tile([128, KT, G4], bf16)
            U_sb = const.tile([128, KT, G4], bf16)
            bias_sb = const.tile([128, MT], f32)
            nc.gpsimd.dma_start(W_sb[:], W[:])
            nc.gpsimd.dma_start(U_sb[:], U[:])
            nc.gpsimd.dma_start(bias_sb[:], biasT[:])

            xwT = dram.tile([128, MT, C], f32)

            # ---- Phase 1: xwT[:, m, t*8+b] = (x_t @ W + bias)^T ----
            for ch in range(T // CHUNK):
                cols = slice(ch * CHUNK * BL, (ch + 1) * CHUNK * BL)
                xtc = xtc_pool.tile([128, KT, CHUNK * BL], bf16)
                nc.gpsimd.dma_start(xtc[:], xT[:, :, cols])
                for m in range(MT):
                    ps = pre_ps.tile([128, CHUNK * BL], f32)
                    for k in range(KT):
                        nc.tensor.matmul(
                            ps[:],
                            W_sb[:, k, m * 128:(m + 1) * 128],
                            xtc[:, k, :],
                            start=(k == 0),
                            stop=(k == KT - 1),
                        )
                    xwc = xwc_pool.tile([128, CHUNK * BL], f32)
                    nc.scalar.activation(
                        xwc[:], ps[:], AF.Identity, bias=bias_sb[:, m:m + 1]
                    )
                    nc.gpsimd.dma_start(xwT[:, m, cols], xwc[:])

            # ---- Phase 2: recurrence ----
            hT = state.tile([128, KT * BL], bf16)   # col = k*8+b
            c_st = state.tile([128, KT * BL], f32)
            nc.vector.memset(hT[:], 0.0)
            nc.vector.memset(c_st[:], 0.0)

            def macro_body(c0, unroll):
                for u in range(unroll):
                    base = c0 + u * (MACRO * BL)
                    stage = stage_pool.tile([128, KT, MACRO * BL], f32)
                    xwm = xw_pool.tile([128, MT, MACRO * BL], f32)
                    nc.gpsimd.dma_start(
                        xwm[:], xwT[:, :, bass.ds(base, MACRO * BL)]
                    )
                    for s in range(MACRO):
                        xw = xwm[:, :, s * BL:(s + 1) * BL]
                        ps = g_ps.tile([128, MT * BL], f32)  # col = m*8+b
                        for m in range(MT):
                            for k in range(KT):
                                nc.tensor.matmul(
                                    ps[:, m * BL:(m + 1) * BL],
                                    U_sb[:, k, m * 128:(m + 1) * 128],
                                    hT[:, k * BL:(k + 1) * BL],
                                    start=(k == 0),
                                    stop=(k == KT - 1),
                                )
                        gs = work.tile([128, MT * BL], f32, tag="gs")
                        nc.vector.tensor_add(
                            gs[:].rearrange("p (m b) -> p m b", m=MT),
                            ps[:].rearrange("p (m b) -> p m b", m=MT),
                            xw,
                        )
                        act = work.tile([128, MT * BL], f32, tag="act")
                        # m 0-3=i, 4-7=f, 8-11=g, 12-15=o (cols of 32 each)
                        nc.scalar.activation(act[:, 0:64], gs[:, 0:64], AF.Sigmoid)
                        nc.scalar.activation(act[:, 64:96], gs[:, 64:96], AF.Tanh)
                        nc.scalar.activation(act[:, 96:128], gs[:, 96:128], AF.Sigmoid)
                        ig = work.tile([128, KT * BL], f32, tag="ig")
                        nc.vector.tensor_mul(ig[:], act[:, 0:32], act[:, 64:96])
                        nc.vector.tensor_mul(c_st[:], act[:, 32:64], c_st[:])
                        nc.vector.tensor_add(c_st[:], c_st[:], ig[:])
                        tc_t = work.tile([128, KT * BL], f32, tag="tc")
                        nc.scalar.activation(tc_t[:], c_st[:], AF.Tanh)
                        hslot = stage[:, :, s * BL:(s + 1) * BL]
                        nc.vector.tensor_mul(
                            hslot,
                            act[:, 96:128].rearrange("p (k b) -> p k b", k=KT),
                            tc_t[:].rearrange("p (k b) -> p k b", k=KT),
                        )
                        nc.vector.tensor_copy(
                            hT[:].rearrange("p (k b) -> p k b", k=KT), hslot
                        )
                    nc.gpsimd.dma_start(
                        hT_out[:, :, bass.ds(base, MACRO * BL)], stage[:]
                    )

            tc.For_i_unrolled_general(
                start=0, end=C, step=MACRO * BL,
                unrollable_body=macro_body, max_unroll=1,
                hint_engines=(mybir.EngineType.PE,),
            )
    nc.finalize()
    return nc


def kernel(x, W, U, bias):
    import concourse.bass as bass
    import concourse.bacc as bacc
    import concourse.tile as tile
    import concourse.mybir as mybir
    from concourse.bass_utils import run_bass_kernel_spmd

    x = np.asarray(x, np.float32)
    W = np.asarray(W, np.float32)
    U = np.asarray(U, np.float32)
    bias = np.asarray(bias, np.float32)

    nc = build(bacc.Bacc("TRN2", target_bir_lowering=False, num_devices=NC), bass, tile, mybir)

    Wt = np.ascontiguousarray(W.reshape(KT, 128, G4).transpose(1, 0, 2)).astype(ml_dtypes.bfloat16)
    Ut = np.ascontiguousarray(
        U.reshape(KT, 128, G4).transpose(1, 0, 2)
    ).astype(ml_dtypes.bfloat16)
    bt = np.ascontiguousarray(bias.reshape(MT, 128).T)

    in_maps = []
    for i in range(NC):
        xl = x[i * BL:(i + 1) * BL]                     # [8, 1024, 512]
        xTl = np.ascontiguousarray(
            xl.transpose(2, 1, 0).reshape(KT, 128, C)   # [512, T, 8]->[4,128,C]
        ).transpose(1, 0, 2)
        in_maps.append({
            "xT": np.ascontiguousarray(xTl).astype(ml_dtypes.bfloat16),
            "W": Wt, "U": Ut, "biasT": bt,
        })

    import os
    trace = bool(os.environ.get("LSTM_TRACE"))
    res = run_bass_kernel_spmd(
        nc, in_maps, core_ids=list(range(NC)), trace=trace
    )
    if trace and res.exec_time_ns is not None:
        print(f"HW exec time: {res.exec_time_ns} ns")
        print("trace:", (res.instructions_and_trace or (None, None))[1])
    out = np.empty((B, T, H), np.float32)
    for i in range(NC):
        ho = res.results[i]["hT_out"]                   # [128, 4, C]
        out[i * BL:(i + 1) * BL] = (
            ho.reshape(128, KT, T, BL).transpose(3, 2, 1, 0).reshape(BL, T, H)
        )
    return out



# revision 5
# speedup vs baseline: 1.5621x; 1.5621x over previous
"""Trainium2 Bass kernel for CustomLSTM: B=64, T=1024, I=H=512.

Sharding: data-parallel over batch, 8 sequences per core on 8 cores.
Transposed on-device layout throughout (gate/hidden dims on SBUF partitions,
(tile, batch) on the free dim) so elementwise runs on all 128 lanes and h^T
feeds the next step's matmul directly with zero transposes.

v2 structure (single fused pass, no DRAM xw round-trip):
- Phase-1 (x@W+bias) is computed chunk-by-chunk (32 steps) into SBUF in bf16,
  interleaved into the recurrence's PE idle windows (one W m-tile per 2 steps).
- Per step, xw_t is injected into PSUM via identity matmuls (start=True), then
  the 64 U.h matmuls accumulate on top (start=False). This removes the
  per-step DVE ADD from the critical chain; sigmois/tanh read PSUM directly.
- h is written once per step as bf16 into a 33-slot stage buffer; the next
  step's matmuls use the previous slot as the moving operand (no copy/cast).
  Macro boundaries chain by reading the previous stage tile's last slot.
- m-tile order i,f,g,o: the c-path activations/muls overlap the o matmuls.
"""

import numpy as np
import ml_dtypes

B, T, I, H = 64, 1024, 512, 512
NC = 8            # cores
BL = B // NC      # 8 sequences per core
G4 = 4 * H        # 2048 gate dim
KT = I // 128     # 4 contraction tiles (and hidden quarters)
MT = G4 // 128    # 16 gate m-tiles: 0-3=i, 4-7=f, 8-11=g, 12-15=o
MACRO = 32        # timesteps per macro block
NMAC = T // MACRO # 32 macro blocks
XROWS = T + 2 * MACRO  # xT2 padded rows (prefetch overrun)


def build(nc, bass, tile, mybir):
    f32, bf16 = mybir.dt.float32, mybir.dt.bfloat16
    AF = mybir.ActivationFunctionType

    xT2 = nc.dram_tensor("xT2", [128, XROWS, KT * BL], bf16, kind="ExternalInput")
    W = nc.dram_tensor("W", [128, KT, G4], bf16, kind="ExternalInput")
    U = nc.dram_tensor("U", [128, KT, G4], bf16, kind="ExternalInput")
    biasT = nc.dram_tensor("biasT", [128, MT], f32, kind="ExternalInput")
    eye = nc.dram_tensor("eye", [128, 128], bf16, kind="ExternalInput")
    hT_out = nc.dram_tensor("hT_out", [128, T, KT * BL], bf16, kind="ExternalOutput")

    SL = KT * BL  # 32: (k, b) columns of h / c state

    with tile.TileContext(nc) as tc:
        with (
            tc.tile_pool(name="const", bufs=1) as const,
            tc.tile_pool(name="pre_ps", bufs=2, space="PSUM") as pre_ps,
            tc.tile_pool(name="g_ps", bufs=2, space="PSUM") as g_ps,
            tc.tile_pool(name="work", bufs=2) as work,
        ):
            W_sb = const.tile([128, KT, G4], bf16)
            U_sb = const.tile([128, KT, G4], bf16)
            bias_sb = const.tile([128, MT], f32)
            eye_sb = const.tile([128, 128], bf16)
            c_st = const.tile([128, SL], f32)
            h0 = const.tile([128, SL], bf16)
            # Explicit ping-pong buffers for cross-macro pipelines (persistent
            # tiles -> fixed addresses, required inside the hardware loop).
            chunks = [const.tile([128, MACRO, 128], bf16, name=f"chunk{i}")
                      for i in range(2)]
            xtcs = [const.tile([128, KT, MACRO * BL], bf16, name=f"xtc{i}")
                    for i in range(2)]
            stages = [const.tile([128, MACRO + 1, SL], bf16, name=f"stg{i}")
                      for i in range(2)]
            nc.gpsimd.dma_start(W_sb[:], W[:])
            nc.gpsimd.dma_start(U_sb[:], U[:])
            nc.gpsimd.dma_start(bias_sb[:], biasT[:])
            nc.gpsimd.dma_start(eye_sb[:], eye[:])
            nc.vector.memset(c_st[:], 0.0)
            nc.vector.memset(h0[:], 0.0)

            def fetch_x(row0, xtc):
                """DMA one chunk of x rows into a k-major xtc tile."""
                for k in range(KT):
                    nc.gpsimd.dma_start(
                        xtc[:, k, :].rearrange("p (t b) -> p t b", t=MACRO),
                        xT2[:, bass.ds(row0, MACRO), k * BL:(k + 1) * BL],
                    )

            def phase1_mtile(xtc, chunk, m):
                """xw for one W m-tile over a 32-step chunk -> chunk SBUF."""
                ps = pre_ps.tile([128, MACRO * BL], f32)
                for k in range(KT):
                    nc.tensor.matmul(
                        ps[:],
                        W_sb[:, k, m * 128:(m + 1) * 128],
                        xtc[:, k, :],
                        start=(k == 0),
                        stop=(k == KT - 1),
                    )
                nc.scalar.activation(
                    chunk[:, :, m * BL:(m + 1) * BL],
                    ps[:].rearrange("p (t b) -> p t b", t=MACRO),
                    AF.Identity,
                    bias=bias_sb[:, m:m + 1],
                )

            def step(chunk, stage, prev_h, s):
                """One recurrence step; h input = prev_h AP, h out -> stage."""
                ps_if = g_ps.tile([128, 8 * BL], f32, tag="if")
                ps_g = g_ps.tile([128, 4 * BL], f32, tag="g")
                ps_o = g_ps.tile([128, 4 * BL], f32, tag="o")
                # xw injection (no h dependency; runs during previous tail)
                nc.tensor.matmul(ps_if[:], eye_sb[:], chunk[:, s, 0:64],
                                 start=True, stop=False)
                nc.tensor.matmul(ps_g[:], eye_sb[:], chunk[:, s, 64:96],
                                 start=True, stop=False)
                nc.tensor.matmul(ps_o[:], eye_sb[:], chunk[:, s, 96:128],
                                 start=True, stop=False)
                # U.h accumulation, m-tile order i,f,g,o
                for m in range(MT):
                    if m < 8:
                        dst = ps_if[:, m * BL:(m + 1) * BL]
                    elif m < 12:
                        dst = ps_g[:, (m - 8) * BL:(m - 7) * BL]
                    else:
                        dst = ps_o[:, (m - 12) * BL:(m - 11) * BL]
                    for k in range(KT):
                        nc.tensor.matmul(
                            dst,
                            U_sb[:, k, m * 128:(m + 1) * 128],
                            prev_h[:, k * BL:(k + 1) * BL],
                            start=False,
                            stop=(k == KT - 1),
                        )
                act_if = work.tile([128, 8 * BL], f32, tag="aif")
                act_g = work.tile([128, 4 * BL], f32, tag="ag")
                act_o = work.tile([128, 4 * BL], f32, tag="ao")
                nc.scalar.activation(act_if[:], ps_if[:], AF.Sigmoid)
                nc.scalar.activation(act_g[:], ps_g[:], AF.Tanh)
                nc.scalar.activation(act_o[:], ps_o[:], AF.Sigmoid)
                fc = work.tile([128, SL], f32, tag="fc")
                ig = work.tile([128, SL], f32, tag="ig")
                nc.vector.tensor_mul(fc[:], act_if[:, SL:2 * SL], c_st[:])
                nc.vector.tensor_mul(ig[:], act_if[:, 0:SL], act_g[:])
                nc.vector.tensor_add(c_st[:], fc[:], ig[:])
                tc_t = work.tile([128, SL], f32, tag="tc")
                nc.scalar.activation(tc_t[:], c_st[:], AF.Tanh)
                nc.vector.tensor_mul(stage[:, s + 1, :], act_o[:], tc_t[:])

            def emit_macro(iv, par, first=False, last=False):
                """One macro: steps on chunks[par], phase-1 for the next chunk
                into chunks[1-par] (reading xtcs[1-par]), prefetch x for the
                chunk after that into xtcs[par]."""
                chunk = chunks[par]
                stage = stages[par]
                prev_stage = stages[1 - par]
                if not last:
                    fetch_x(iv + 2 * MACRO, xtcs[par])
                for s in range(MACRO):
                    if s == 0:
                        prev_h = h0[:] if first else prev_stage[:, MACRO, :]
                    else:
                        prev_h = stage[:, s, :]
                    step(chunk, stage, prev_h, s)
                    if (not last) and s % 2 == 0:
                        phase1_mtile(xtcs[1 - par], chunks[1 - par], s // 2)
                nc.gpsimd.dma_start(
                    hT_out[:, bass.ds(iv, MACRO), :],
                    stage[:, 1:MACRO + 1, :],
                )

            # Prologue: x rows 0:32 and 32:64, then chunk 0 serially.
            fetch_x(0, xtcs[0])
            fetch_x(MACRO, xtcs[1])
            for m in range(MT):
                phase1_mtile(xtcs[0], chunks[0], m)

            emit_macro(0, 0, first=True)

            def loop_body(iv, unroll):
                for u in range(unroll):
                    emit_macro(iv + u * MACRO, (1 + u) % 2)

            tc.For_i_unrolled_general(
                start=MACRO, end=(NMAC - 1) * MACRO, step=MACRO,
                unrollable_body=loop_body, max_unroll=2,
                hint_engines=(mybir.EngineType.PE,),
            )

            emit_macro((NMAC - 1) * MACRO, (NMAC - 1) % 2, last=True)
    nc.finalize()
    return nc


def kernel(x, W, U, bias):
    import concourse.bass as bass
    import concourse.bacc as bacc
    import concourse.tile as tile
    import concourse.mybir as mybir
    from concourse.bass_utils import run_bass_kernel_spmd

    x = np.asarray(x, np.float32)
    W = np.asarray(W, np.float32)
    U = np.asarray(U, np.float32)
    bias = np.asarray(bias, np.float32)

    nc = build(bacc.Bacc("TRN2", target_bir_lowering=False, num_devices=NC),
               bass, tile, mybir)

    Wt = np.ascontiguousarray(
        W.reshape(KT, 128, G4).transpose(1, 0, 2)
    ).astype(ml_dtypes.bfloat16)
    Ut = np.ascontiguousarray(
        U.reshape(KT, 128, G4).transpose(1, 0, 2)
    ).astype(ml_dtypes.bfloat16)
    bt = np.ascontiguousarray(bias.reshape(MT, 128).T)
    ey = np.eye(128, dtype=np.float32).astype(ml_dtypes.bfloat16)

    in_maps = []
    for i in range(NC):
        xl = x[i * BL:(i + 1) * BL]                      # [8, 1024, 512]
        # xT2[p, t, k*8+b] = x[b, t, k*128+p]
        xt = xl.reshape(BL, T, KT, 128).transpose(3, 1, 2, 0).reshape(128, T, KT * BL)
        xp = np.zeros((128, XROWS, KT * BL), np.float32)
        xp[:, :T] = xt
        in_maps.append({
            "xT2": xp.astype(ml_dtypes.bfloat16),
            "W": Wt, "U": Ut, "biasT": bt, "eye": ey,
        })

    import os
    trace = bool(os.environ.get("LSTM_TRACE"))
    res = run_bass_kernel_spmd(
        nc, in_maps, core_ids=list(range(NC)), trace=trace
    )
    if trace and res.exec_time_ns is not None:
        print(f"HW exec time: {res.exec_time_ns} ns")
        print("trace:", (res.instructions_and_trace or (None, None))[1])
    out = np.empty((B, T, H), np.float32)
    for i in range(NC):
        ho = np.asarray(res.results[i]["hT_out"], dtype=np.float32)  # [128,1024,32]
        # out[b, t, k*128+p] = ho[p, t, k*8+b]
        out[i * BL:(i + 1) * BL] = (
            ho.reshape(128, T, KT, BL).transpose(3, 1, 2, 0).reshape(BL, T, H)
        )
    return out


# revision 9
# speedup vs baseline: 1.5650x; 1.0019x over previous
"""Trainium2 Bass kernel for CustomLSTM: B=64, T=1024, I=H=512.

Sharding: data-parallel over batch, 8 sequences per core on 8 cores.
Transposed on-device layout throughout (gate/hidden dims on SBUF partitions,
(tile, batch) on the free dim) so elementwise runs on all 128 lanes and h^T
feeds the next step's matmul directly with zero transposes.

v2 structure (single fused pass, no DRAM xw round-trip):
- Phase-1 (x@W+bias) is computed chunk-by-chunk (32 steps) into SBUF in bf16,
  interleaved into the recurrence's PE idle windows (one W m-tile per 2 steps).
- Per step, xw_t is injected into PSUM via identity matmuls (start=True), then
  the 64 U.h matmuls accumulate on top (start=False). This removes the
  per-step DVE ADD from the critical chain; sigmois/tanh read PSUM directly.
- h is written once per step as bf16 into a 33-slot stage buffer; the next
  step's matmuls use the previous slot as the moving operand (no copy/cast).
  Macro boundaries chain by reading the previous stage tile's last slot.
- m-tile order i,f,g,o: the c-path activations/muls overlap the o matmuls.
"""

import numpy as np
import ml_dtypes

B, T, I, H = 64, 1024, 512, 512
NC = 8            # cores
BL = B // NC      # 8 sequences per core
G4 = 4 * H        # 2048 gate dim
KT = I // 128     # 4 contraction tiles (and hidden quarters)
MT = G4 // 128    # 16 gate m-tiles: 0-3=i, 4-7=f, 8-11=g, 12-15=o
MACRO = 32        # timesteps per macro block
NMAC = T // MACRO # 32 macro blocks
XROWS = T + 2 * MACRO  # xT2 padded rows (prefetch overrun)


def build(nc, bass, tile, mybir):
    f32, bf16 = mybir.dt.float32, mybir.dt.bfloat16
    AF = mybir.ActivationFunctionType

    xT2 = nc.dram_tensor("xT2", [128, XROWS, KT * BL], bf16, kind="ExternalInput")
    W = nc.dram_tensor("W", [128, KT, G4], bf16, kind="ExternalInput")
    U = nc.dram_tensor("U", [128, KT, G4], bf16, kind="ExternalInput")
    biasT = nc.dram_tensor("biasT", [128, MT], f32, kind="ExternalInput")
    eye = nc.dram_tensor("eye", [128, 128], bf16, kind="ExternalInput")
    hT_out = nc.dram_tensor("hT_out", [128, T, KT * BL], bf16, kind="ExternalOutput")

    SL = KT * BL  # 32: (k, b) columns of h / c state

    with tile.TileContext(nc) as tc:
        with (
            tc.tile_pool(name="const", bufs=1) as const,
            tc.tile_pool(name="pre_ps", bufs=2, space="PSUM") as pre_ps,
            tc.tile_pool(name="g_ps", bufs=2, space="PSUM") as g_ps,
            tc.tile_pool(name="work", bufs=2) as work,
        ):
            W_sb = const.tile([128, KT, G4], bf16)
            U_sb = const.tile([128, KT, G4], bf16)
            bias_sb = const.tile([128, MT], f32)
            eye_sb = const.tile([128, 128], bf16)
            c_st = const.tile([128, SL], f32)
            h0 = const.tile([128, SL], bf16)
            # Explicit ping-pong buffers for cross-macro pipelines (persistent
            # tiles -> fixed addresses, required inside the hardware loop).
            chunks = [const.tile([128, MACRO, 128], bf16, name=f"chunk{i}")
                      for i in range(2)]
            xtcs = [const.tile([128, KT, MACRO * BL], bf16, name=f"xtc{i}")
                    for i in range(2)]
            stages = [const.tile([128, MACRO + 1, SL], bf16, name=f"stg{i}")
                      for i in range(2)]
            nc.gpsimd.dma_start(W_sb[:], W[:])
            nc.gpsimd.dma_start(U_sb[:], U[:])
            nc.gpsimd.dma_start(bias_sb[:], biasT[:])
            nc.gpsimd.dma_start(eye_sb[:], eye[:])
            nc.vector.memset(c_st[:], 0.0)
            nc.vector.memset(h0[:], 0.0)

            def fetch_x(row0, xtc):
                """DMA one chunk of x rows into a k-major xtc tile."""
                for k in range(KT):
                    nc.gpsimd.dma_start(
                        xtc[:, k, :].rearrange("p (t b) -> p t b", t=MACRO),
                        xT2[:, bass.ds(row0, MACRO), k * BL:(k + 1) * BL],
                    )

            def phase1_mtile(xtc, chunk, m):
                """xw for one W m-tile over a 32-step chunk -> chunk SBUF."""
                # Full-bank tile: PSUM hazards are bank-granular; sharing a
                # bank with the per-step gate tiles serializes the pipeline.
                ps = pre_ps.tile([128, 512], f32, name="pps")[:, 0:MACRO * BL]
                for k in range(KT):
                    nc.tensor.matmul(
                        ps[:],
                        W_sb[:, k, m * 128:(m + 1) * 128],
                        xtc[:, k, :],
                        start=(k == 0),
                        stop=(k == KT - 1),
                    )
                nc.scalar.activation(
                    chunk[:, :, m * BL:(m + 1) * BL],
                    ps[:].rearrange("p (t b) -> p t b", t=MACRO),
                    AF.Identity,
                    bias=bias_sb[:, m:m + 1],
                )

            def step(chunk, stage, prev_h, s):
                """One recurrence step; h input = prev_h AP, h out -> stage."""
                ps_if = g_ps.tile([128, 512], f32, tag="if", name="psif")[:, 0:8 * BL]
                ps_g = g_ps.tile([128, 512], f32, tag="g", name="psg")[:, 0:4 * BL]
                ps_o = g_ps.tile([128, 512], f32, tag="o", name="pso")[:, 0:4 * BL]
                # xw injection (no h dependency; runs during previous tail)
                nc.tensor.matmul(ps_if[:], eye_sb[:], chunk[:, s, 0:64],
                                 start=True, stop=False)
                nc.tensor.matmul(ps_g[:], eye_sb[:], chunk[:, s, 64:96],
                                 start=True, stop=False)
                nc.tensor.matmul(ps_o[:], eye_sb[:], chunk[:, s, 96:128],
                                 start=True, stop=False)
                # U.h accumulation, m-tile order i,f,g,o
                for m in range(MT):
                    if m < 8:
                        dst = ps_if[:, m * BL:(m + 1) * BL]
                    elif m < 12:
                        dst = ps_g[:, (m - 8) * BL:(m - 7) * BL]
                    else:
                        dst = ps_o[:, (m - 12) * BL:(m - 11) * BL]
                    for k in range(KT):
                        nc.tensor.matmul(
                            dst,
                            U_sb[:, k, m * 128:(m + 1) * 128],
                            prev_h[:, k * BL:(k + 1) * BL],
                            start=False,
                            stop=(k == KT - 1),
                        )
                act_if = work.tile([128, 8 * BL], f32, tag="aif")
                act_g = work.tile([128, 4 * BL], f32, tag="ag")
                act_o = work.tile([128, 4 * BL], f32, tag="ao")
                nc.scalar.activation(act_if[:], ps_if[:], AF.Sigmoid)
                nc.scalar.activation(act_g[:], ps_g[:], AF.Tanh)
                nc.scalar.activation(act_o[:], ps_o[:], AF.Sigmoid)
                fc = work.tile([128, SL], f32, tag="fc")
                ig = work.tile([128, SL], f32, tag="ig")
                nc.vector.tensor_mul(fc[:], act_if[:, SL:2 * SL], c_st[:])
                nc.vector.tensor_mul(ig[:], act_if[:, 0:SL], act_g[:])
                nc.vector.tensor_add(c_st[:], fc[:], ig[:])
                tc_t = work.tile([128, SL], f32, tag="tc")
                nc.scalar.activation(tc_t[:], c_st[:], AF.Tanh)
                nc.vector.tensor_mul(stage[:, s + 1, :], act_o[:], tc_t[:])

            def emit_macro(iv, par, first=False, last=False):
                """One macro: steps on chunks[par], phase-1 for the next chunk
                into chunks[1-par] (reading xtcs[1-par]), prefetch x for the
                chunk after that into xtcs[par]."""
                chunk = chunks[par]
                stage = stages[par]
                prev_stage = stages[1 - par]
                if not last:
                    fetch_x(iv + 2 * MACRO, xtcs[par])
                for s in range(MACRO):
                    if s == 0:
                        prev_h = h0[:] if first else prev_stage[:, MACRO, :]
                    else:
                        prev_h = stage[:, s, :]
                    step(chunk, stage, prev_h, s)
                    if (not last) and s % 2 == 0:
                        phase1_mtile(xtcs[1 - par], chunks[1 - par], s // 2)
                nc.gpsimd.dma_start(
                    hT_out[:, bass.ds(iv, MACRO), :],
                    stage[:, 1:MACRO + 1, :],
                )

            # Prologue: x rows 0:32 and 32:64, then chunk 0 serially.
            fetch_x(0, xtcs[0])
            fetch_x(MACRO, xtcs[1])
            for m in range(MT):
                phase1_mtile(xtcs[0], chunks[0], m)

            emit_macro(0, 0, first=True)

            def loop_body(iv, unroll):
                for u in range(unroll):
                    emit_macro(iv + u * MACRO, (1 + u) % 2)

            tc.For_i_unrolled_general(
                start=MACRO, end=(NMAC - 1) * MACRO, step=MACRO,
                unrollable_body=loop_body, max_unroll=2,
                hint_engines=(mybir.EngineType.PE,),
            )

            emit_macro((NMAC - 1) * MACRO, (NMAC - 1) % 2, last=True)
    nc.finalize()
    return nc


def kernel(x, W, U, bias):
    import concourse.bass as bass
    import concourse.bacc as bacc
    import concourse.tile as tile
    import concourse.mybir as mybir
    from concourse.bass_utils import run_bass_kernel_spmd

    x = np.asarray(x, np.float32)
    W = np.asarray(W, np.float32)
    U = np.asarray(U, np.float32)
    bias = np.asarray(bias, np.float32)

    nc = build(bacc.Bacc("TRN2", target_bir_lowering=False, num_devices=NC),
               bass, tile, mybir)

    Wt = np.ascontiguousarray(
        W.reshape(KT, 128, G4).transpose(1, 0, 2)
    ).astype(ml_dtypes.bfloat16)
    Ut = np.ascontiguousarray(
        U.reshape(KT, 128, G4).transpose(1, 0, 2)
    ).astype(ml_dtypes.bfloat16)
    bt = np.ascontiguousarray(bias.reshape(MT, 128).T)
    ey = np.eye(128, dtype=np.float32).astype(ml_dtypes.bfloat16)

    in_maps = []
    for i in range(NC):
        xl = x[i * BL:(i + 1) * BL]                      # [8, 1024, 512]
        # xT2[p, t, k*8+b] = x[b, t, k*128+p]
        xt = xl.reshape(BL, T, KT, 128).transpose(3, 1, 2, 0).reshape(128, T, KT * BL)
        xp = np.zeros((128, XROWS, KT * BL), np.float32)
        xp[:, :T] = xt
        in_maps.append({
            "xT2": xp.astype(ml_dtypes.bfloat16),
            "W": Wt, "U": Ut, "biasT": bt, "eye": ey,
        })

    import os
    trace = bool(os.environ.get("LSTM_TRACE"))
    res = run_bass_kernel_spmd(
        nc, in_maps, core_ids=list(range(NC)), trace=trace
    )
    if trace and res.exec_time_ns is not None:
        print(f"HW exec time: {res.exec_time_ns} ns")
        print("trace:", (res.instructions_and_trace or (None, None))[1])
    out = np.empty((B, T, H), np.float32)
    for i in range(NC):
        ho = np.asarray(res.results[i]["hT_out"], dtype=np.float32)  # [128,1024,32]
        # out[b, t, k*128+p] = ho[p, t, k*8+b]
        out[i * BL:(i + 1) * BL] = (
            ho.reshape(128, T, KT, BL).transpose(3, 1, 2, 0).reshape(BL, T, H)
        )
    return out
